# revision 1
# baseline (speedup 1.0000x reference)
"""Block-diagonal cross-attention + MLP for trn2, 8-core data-parallel.

v2: 16 graphs/core padded to GCAP=128 rows/side, processed in PAIRS:
  - scores psum tile [128,512] = {SS_a, ST_a, SS_b, ST_b}; one exp ACT op
  - O psum [128,260] = 4x [nodes, 64 V-cols | rsum]; V matmul lhsT = E slice
  - normalize per-partition (recip of the rsum col), PE-transpose into packed
    psum ([0:64)=srcT, [64:128)=tarT via tile_position=(0,64)), evict fused
    with +x residual (packed xT) into eT_packed [128, 2048]
  - MLP on eT_packed with block-diag weights [128,128]: both sides at once
Output: one [128,2048] tensor per core; host unpads/reassembles.
"""

from contextlib import ExitStack

import numpy as np

N_NODES = 8192
D = 64
G = 128
N_CORES = 8
GPC = G // N_CORES          # graphs per core = 16
GCAP = 128                  # padded nodes per graph per side
ROWS = GPC * GCAP           # 2048 padded rows per core
VW = D + 1                  # v-rows width incl. mask column
NPAIR = GPC // 2

_PROGRAM_CACHE = {}


def _build_program(stop_after=None):
    import concourse.bass as bass
    import concourse.tile as tile
    from concourse import bacc, mybir

    fp32 = mybir.dt.float32
    nc = bacc.Bacc("TRN2", target_bir_lowering=False, debug=False)

    xsT = nc.declare_dram_parameter("xsT", [D, ROWS], fp32, isOutput=False)
    xtT = nc.declare_dram_parameter("xtT", [D, ROWS], fp32, isOutput=False)
    vs = nc.declare_dram_parameter("vs", [GCAP, GPC * VW], fp32, isOutput=False)
    vt = nc.declare_dram_parameter("vt", [GCAP, GPC * VW], fp32, isOutput=False)
    w1bd = nc.declare_dram_parameter("w1bd", [2 * D, 2 * D], fp32, isOutput=False)
    b1bd = nc.declare_dram_parameter("b1bd", [2 * D, 1], fp32, isOutput=False)
    w2bd = nc.declare_dram_parameter("w2bd", [2 * D, 2 * D], fp32, isOutput=False)
    b2bd = nc.declare_dram_parameter("b2bd", [2 * D, 1], fp32, isOutput=False)
    ident = nc.declare_dram_parameter("ident", [GCAP, GCAP], fp32, isOutput=False)
    outp = nc.declare_dram_parameter("outp", [2 * D, ROWS], fp32, isOutput=True)

    AF = mybir.ActivationFunctionType
    ALU = mybir.AluOpType

    with tile.TileContext(nc) as tc, ExitStack() as ctx:
        singles = ctx.enter_context(tc.tile_pool(name="singles", bufs=1))
        epool = ctx.enter_context(tc.tile_pool(name="epool", bufs=3))
        work = ctx.enter_context(tc.tile_pool(name="work", bufs=3))

        sb_xsT = singles.tile([D, ROWS], fp32, tag="xsT")
        sb_xtT = singles.tile([D, ROWS], fp32, tag="xtT")
        sb_xp = singles.tile([2 * D, ROWS], fp32, tag="xp")
        sb_vs = singles.tile([GCAP, GPC * VW], fp32, tag="vs")
        sb_vt = singles.tile([GCAP, GPC * VW], fp32, tag="vt")
        sb_w1 = singles.tile([2 * D, 2 * D], fp32, tag="w1")
        sb_b1 = singles.tile([2 * D, 1], fp32, tag="b1")
        sb_w2 = singles.tile([2 * D, 2 * D], fp32, tag="w2")
        sb_b2 = singles.tile([2 * D, 1], fp32, tag="b2")
        sb_id = singles.tile([GCAP, GCAP], fp32, tag="ident")
        sb_eT = singles.tile([2 * D, ROWS], fp32, tag="eT")
        sb_h = singles.tile([2 * D, ROWS], fp32, tag="h")
        sb_out = singles.tile([2 * D, ROWS], fp32, tag="out")

        nc.sync.dma_start(out=sb_xsT, in_=xsT[:, :])
        nc.sync.dma_start(out=sb_xtT, in_=xtT[:, :])
        nc.sync.dma_start(out=sb_vs, in_=vs[:, :])
        nc.sync.dma_start(out=sb_vt, in_=vt[:, :])
        nc.sync.dma_start(out=sb_w1, in_=w1bd[:, :])
        nc.sync.dma_start(out=sb_b1, in_=b1bd[:, :])
        nc.sync.dma_start(out=sb_w2, in_=w2bd[:, :])
        nc.sync.dma_start(out=sb_b2, in_=b2bd[:, :])
        nc.sync.dma_start(out=sb_id, in_=ident[:, :])
        # packed xT for the fused evict+residual (on-chip copies)
        nc.sync.dma_start(out=sb_xp[0:D, :], in_=sb_xsT)
        nc.sync.dma_start(out=sb_xp[D:2 * D, :], in_=sb_xtT)

        with tc.tile_pool(name="ps_sc", bufs=3, space="PSUM") as ps_sc, \
             tc.tile_pool(name="ps_ot", bufs=3, space="PSUM") as ps_ot, \
             tc.tile_pool(name="ps_m", bufs=2, space="PSUM") as ps_m:
            for k in range(NPAIR):
                a, b = 2 * k, 2 * k + 1
                ca, cb = a * GCAP, b * GCAP
                va, vb = a * VW, b * VW
                xs_a = sb_xsT[:, ca:ca + GCAP]
                xt_a = sb_xtT[:, ca:ca + GCAP]
                xs_b = sb_xsT[:, cb:cb + GCAP]
                xt_b = sb_xtT[:, cb:cb + GCAP]

                if stop_after == "dma":
                    continue
                sc = ps_sc.tile([GCAP, 4 * GCAP], fp32, tag="sc")
                nc.tensor.matmul(sc[:, 0:128], xs_a, xt_a, start=True, stop=True)
                nc.tensor.matmul(sc[:, 128:256], xt_a, xs_a, start=True, stop=True)
                nc.tensor.matmul(sc[:, 256:384], xs_b, xt_b, start=True, stop=True)
                nc.tensor.matmul(sc[:, 384:512], xt_b, xs_b, start=True, stop=True)

                et = epool.tile([GCAP, 4 * GCAP], fp32, tag="E")
                nc.scalar.activation(out=et, in_=sc, func=AF.Exp)
                if stop_after == "scores":
                    nc.sync.dma_start(out=outp[:, ca:ca + GCAP], in_=et[:, 0:GCAP])
                    continue

                o = ps_ot.tile([GCAP, 4 * VW], fp32, tag="ot")
                nc.tensor.matmul(o[:, 0:VW], et[:, 128:256],
                                 sb_vt[:, va:va + VW], start=True, stop=True)
                nc.tensor.matmul(o[:, VW:2 * VW], et[:, 0:128],
                                 sb_vs[:, va:va + VW], start=True, stop=True)
                nc.tensor.matmul(o[:, 2 * VW:3 * VW], et[:, 384:512],
                                 sb_vt[:, vb:vb + VW], start=True, stop=True)
                nc.tensor.matmul(o[:, 3 * VW:4 * VW], et[:, 256:384],
                                 sb_vs[:, vb:vb + VW], start=True, stop=True)

                o3 = o.rearrange("p (q w) -> p q w", q=4)
                rc = work.tile([GCAP, 4], fp32, tag="rc")
                nc.vector.reciprocal(out=rc, in_=o3[:, :, D:D + 1])

                er = work.tile([GCAP, 4 * D], fp32, tag="er")
                for j in range(4):
                    if j % 2 == 0:
                        nc.scalar.mul(er[:, j * D:(j + 1) * D],
                                      o[:, j * VW:j * VW + D], rc[:, j:j + 1])
                    else:
                        nc.vector.tensor_scalar_mul(
                            er[:, j * D:(j + 1) * D],
                            o[:, j * VW:j * VW + D], rc[:, j:j + 1])

                if stop_after == "er":
                    nc.sync.dma_start(out=outp[:, ca:ca + 2 * GCAP], in_=er)
                    continue

                tp = ps_ot.tile([2 * D, 2 * GCAP], fp32, tag="ot")
                # er block layout [src|tar] transposes straight into the
                # packed [srcT; tarT] partition layout
                nc.tensor.transpose(tp[:, 0:GCAP], er[:, 0:2 * D], sb_id)
                nc.tensor.transpose(tp[:, GCAP:2 * GCAP], er[:, 2 * D:4 * D], sb_id)
                # fused evict + residual
                nc.vector.tensor_add(sb_eT[:, ca:ca + 2 * GCAP], tp,
                                     sb_xp[:, ca:ca + 2 * GCAP])

            # ---- packed MLP over [128, 2048] in chunks of 512 ----
            CH = 512
            if stop_after is None:
                for c in range(0, ROWS, CH):
                    hp = ps_m.tile([2 * D, CH], fp32, tag="m")
                    nc.tensor.matmul(hp, sb_w1, sb_eT[:, c:c + CH],
                                     start=True, stop=True)
                    nc.vector.tensor_scalar(
                        out=sb_h[:, c:c + CH], in0=hp, scalar1=sb_b1,
                        scalar2=0.0, op0=ALU.add, op1=ALU.max)
                for c in range(0, ROWS, CH):
                    op2 = ps_m.tile([2 * D, CH], fp32, tag="m")
                    nc.tensor.matmul(op2, sb_w2, sb_h[:, c:c + CH],
                                     start=True, stop=False)
                    # residual folded into psum via identity matmul
                    nc.tensor.matmul(op2, sb_id, sb_eT[:, c:c + CH],
                                     start=False, stop=True)
                    nc.scalar.activation(out=sb_out[:, c:c + CH], in_=op2,
                                         func=AF.Identity, bias=sb_b2, scale=1.0)
                nc.sync.dma_start(out=outp[:, :], in_=sb_out)
            elif stop_after == "attn":
                nc.sync.dma_start(out=outp[:, :], in_=sb_eT)
            elif stop_after == "dma":
                nc.sync.dma_start(out=outp[0:D, :], in_=sb_xsT)

    nc.compile()
    return nc


def _shard_inputs(x_src, batch_src, x_tar, batch_tar, w1, b1, w2, b2):
    """Build per-core padded DRAM images. Returns (in_maps, meta)."""
    bs = np.asarray(batch_src).astype(np.int64)
    bt = np.asarray(batch_tar).astype(np.int64)
    xs = np.asarray(x_src, dtype=np.float32)
    xt = np.asarray(x_tar, dtype=np.float32)

    bnd_s = np.searchsorted(bs, np.arange(G + 1))
    bnd_t = np.searchsorted(bt, np.arange(G + 1))
    cnt_s = np.diff(bnd_s)
    cnt_t = np.diff(bnd_t)
    if cnt_s.max(initial=0) > GCAP or cnt_t.max(initial=0) > GCAP:
        return None, (bnd_s, bnd_t, cnt_s, cnt_t)

    w1a = np.asarray(w1, dtype=np.float32)
    w2a = np.asarray(w2, dtype=np.float32)
    b1a = np.asarray(b1, dtype=np.float32).reshape(D)
    b2a = np.asarray(b2, dtype=np.float32).reshape(D)
    w1bd = np.zeros((2 * D, 2 * D), dtype=np.float32)
    w2bd = np.zeros((2 * D, 2 * D), dtype=np.float32)
    w1bd[:D, :D] = w1a; w1bd[D:, D:] = w1a
    w2bd[:D, :D] = w2a; w2bd[D:, D:] = w2a
    b1bd = np.concatenate([b1a, b1a]).reshape(2 * D, 1)
    b2bd = np.concatenate([b2a, b2a]).reshape(2 * D, 1)
    ident = np.eye(GCAP, dtype=np.float32)

    in_maps = []
    for c in range(N_CORES):
        xs_pad = np.zeros((GPC, GCAP, D), dtype=np.float32)
        xt_pad = np.zeros((GPC, GCAP, D), dtype=np.float32)
        ms = np.zeros((GPC, GCAP), dtype=np.float32)
        mt = np.zeros((GPC, GCAP), dtype=np.float32)
        for i in range(GPC):
            g = c * GPC + i
            ns, nt = cnt_s[g], cnt_t[g]
            xs_pad[i, :ns] = xs[bnd_s[g]:bnd_s[g + 1]]
            xt_pad[i, :nt] = xt[bnd_t[g]:bnd_t[g + 1]]
            ms[i, :ns] = 1.0
            mt[i, :nt] = 1.0
        xsT_img = np.ascontiguousarray(xs_pad.reshape(ROWS, D).T)
        xtT_img = np.ascontiguousarray(xt_pad.reshape(ROWS, D).T)
        vs_img = np.concatenate(
            [xs_pad.transpose(1, 0, 2), ms.T[:, :, None]], axis=2)
        vt_img = np.concatenate(
            [xt_pad.transpose(1, 0, 2), mt.T[:, :, None]], axis=2)
        in_maps.append({
            "xsT": xsT_img,
            "xtT": xtT_img,
            "vs": np.ascontiguousarray(vs_img.reshape(GCAP, GPC * VW)),
            "vt": np.ascontiguousarray(vt_img.reshape(GCAP, GPC * VW)),
            "w1bd": w1bd, "b1bd": b1bd, "w2bd": w2bd, "b2bd": b2bd,
            "ident": ident,
        })
    return in_maps, (bnd_s, bnd_t, cnt_s, cnt_t)


def _numpy_fallback(x_src, batch_src, x_tar, batch_tar, w1, b1, w2, b2):
    bs = np.asarray(batch_src); bt = np.asarray(batch_tar)
    xs = np.asarray(x_src, dtype=np.float64); xt = np.asarray(x_tar, dtype=np.float64)
    mask = bs[:, None] == bt[None, :]

    def attend(q, kv, m):
        s = np.where(m, q @ kv.T, -1.0e9)
        s = s - s.max(axis=1, keepdims=True)
        e = np.exp(s)
        a = e / e.sum(axis=1, keepdims=True)
        out = a @ kv + q
        return np.where(m.any(axis=1, keepdims=True), out, 0.0)

    def mlp(x):
        return np.maximum(x @ w1 + b1, 0.0) @ w2 + b2 + x

    es = mlp(attend(xs, xt, mask))
    et = mlp(attend(xt, xs, mask.T))
    return et.astype(np.float32), es.astype(np.float32)


def kernel(x_src, batch_src, x_tar, batch_tar, w1, b1, w2, b2):
    in_maps, meta = _shard_inputs(
        x_src, batch_src, x_tar, batch_tar, w1, b1, w2, b2)
    if in_maps is None:  # a graph overflowed GCAP; never happens for spec data
        return _numpy_fallback(
            x_src, batch_src, x_tar, batch_tar, w1, b1, w2, b2)
    bnd_s, bnd_t, cnt_s, cnt_t = meta

    import os
    from concourse import bass_utils
    if "nc" not in _PROGRAM_CACHE:
        _PROGRAM_CACHE["nc"] = _build_program()
    nc = _PROGRAM_CACHE["nc"]
    trace = bool(os.environ.get("KERNEL_TRACE"))
    res = bass_utils.run_bass_kernel_spmd(
        nc, in_maps, core_ids=list(range(N_CORES)), trace=trace)
    _PROGRAM_CACHE["last_result"] = res

    embed_src = np.zeros((N_NODES, D), dtype=np.float32)
    embed_tar = np.zeros((N_NODES, D), dtype=np.float32)
    for c in range(N_CORES):
        op = np.asarray(res.results[c]["outp"])  # [128, ROWS]
        o_s = op[0:D]
        o_t = op[D:2 * D]
        for i in range(GPC):
            g = c * GPC + i
            ns, nt = cnt_s[g], cnt_t[g]
            if nt > 0:  # src rows valid only if tar side nonempty
                embed_src[bnd_s[g]:bnd_s[g] + ns] = o_s[:, i * GCAP:i * GCAP + ns].T
            if ns > 0:
                embed_tar[bnd_t[g]:bnd_t[g] + nt] = o_t[:, i * GCAP:i * GCAP + nt].T
    return embed_tar, embed_src



# revision 13
# speedup vs baseline: 1.3151x; 1.3151x over previous
"""Block-diagonal cross-attention + MLP for trn2, 8-core data-parallel.

v3: graphs bin-packed in pairs into 128-row blocks (NB blocks/core).
Cross-graph + padding masking is folded into the score matmul via two
extra contraction rows (graph-code sigma in {+1,-1}): score' = q.k
- 25*(1 - sigma_q*sigma_k), so cross-graph pairs get -50 and padded
columns -25 -> exp ~ 0.  Only exp(SS) is computed on ACT; E^T comes
from a DMA transpose (ST = SS^T exactly).  V-matmul eviction fuses
normalize (divide by the mask-column rowsum) and the +x residual via
scalar_tensor_tensor on DVE/Pool.  er is DMA-transposed straight into
the feature-major MLP input.  MLP uses block-diagonal [128,128]
weights to do both sides at once; residual+bias fused into the final
eviction.  dtypes: fp16 images/weights everywhere except E/V (bf16,
exp can reach e^46 which overflows fp16).
Output: [128, NB*128] fp16 per core; host scatters per graph.
"""

from contextlib import ExitStack

import numpy as np

N_NODES = 8192
D = 64
G = 128
N_CORES = 8
BCAP = 128                  # rows per block
CD = D + 2                  # contraction rows incl. bias rows
VW = D + 1                  # v image width incl. mask column
NEG = 25.0                  # pad bias; cross-graph pairs get -2*NEG

_PROGRAM_CACHE = {}


def _build_program(nb, mlp_ch):
    import concourse.bass as bass
    import concourse.tile as tile
    from concourse import bacc, mybir

    fp32 = mybir.dt.float32
    fp16 = mybir.dt.float16
    bf16 = mybir.dt.bfloat16
    rows = nb * BCAP
    nsb = (nb + 3) // 4           # superblocks of up to 4 blocks
    nc = bacc.Bacc("TRN2", target_bir_lowering=False, debug=False)

    xsT = nc.declare_dram_parameter("xsT", [CD, rows], fp16, isOutput=False)
    xtT = nc.declare_dram_parameter("xtT", [CD, rows], fp16, isOutput=False)
    vs = nc.declare_dram_parameter("vs", [BCAP, nb * VW], bf16, isOutput=False)
    vt = nc.declare_dram_parameter("vt", [BCAP, nb * VW], bf16, isOutput=False)
    vres = nc.declare_dram_parameter("vres", [BCAP, nb * 2 * D], fp16,
                                     isOutput=False)
    w1bd = nc.declare_dram_parameter("w1bd", [2 * D, 2 * D], fp16, isOutput=False)
    b1bd = nc.declare_dram_parameter("b1bd", [2 * D, 1], fp32, isOutput=False)
    w2bd = nc.declare_dram_parameter("w2bd", [2 * D, 2 * D], fp16, isOutput=False)
    b2bd = nc.declare_dram_parameter("b2bd", [2 * D, 1], fp32, isOutput=False)
    outp = nc.declare_dram_parameter("outp", [2 * D, rows], fp16, isOutput=True)

    AF = mybir.ActivationFunctionType
    ALU = mybir.AluOpType

    with tile.TileContext(nc) as tc, ExitStack() as ctx:
        singles = ctx.enter_context(tc.tile_pool(name="singles", bufs=1))
        epool = ctx.enter_context(tc.tile_pool(name="epool", bufs=3))
        tpool = ctx.enter_context(tc.tile_pool(name="tpool", bufs=3))
        rpool = ctx.enter_context(tc.tile_pool(name="rpool", bufs=3))

        sb_xsT = singles.tile([CD, rows], fp16, tag="xsT")
        sb_xtT = singles.tile([CD, rows], fp16, tag="xtT")
        sb_vs = singles.tile([BCAP, nb * VW], bf16, tag="vs")
        sb_vt = singles.tile([BCAP, nb * VW], bf16, tag="vt")
        sb_vres = singles.tile([BCAP, nb * 2 * D], fp16, tag="vres")
        sb_w1 = singles.tile([2 * D, 2 * D], fp16, tag="w1")
        sb_b1 = singles.tile([2 * D, 1], fp32, tag="b1")
        sb_w2 = singles.tile([2 * D, 2 * D], fp16, tag="w2")
        sb_b2 = singles.tile([2 * D, 1], fp32, tag="b2")
        sb_eT = singles.tile([2 * D, rows], fp16, tag="eT")
        sb_h = singles.tile([2 * D, rows], fp16, tag="h")
        sb_out = singles.tile([2 * D, rows], fp16, tag="out")

        nc.sync.dma_start(out=sb_xsT, in_=xsT[:, :])
        nc.sync.dma_start(out=sb_xtT, in_=xtT[:, :])
        nc.sync.dma_start(out=sb_vs, in_=vs[:, :])
        nc.sync.dma_start(out=sb_vt, in_=vt[:, :])
        nc.sync.dma_start(out=sb_vres, in_=vres[:, :])
        nc.sync.dma_start(out=sb_w1, in_=w1bd[:, :])
        nc.sync.dma_start(out=sb_b1, in_=b1bd[:, :])
        nc.sync.dma_start(out=sb_w2, in_=w2bd[:, :])
        nc.sync.dma_start(out=sb_b2, in_=b2bd[:, :])

        with tc.tile_pool(name="ps_sc", bufs=2, space="PSUM") as ps_sc, \
             tc.tile_pool(name="ps_ot", bufs=3, space="PSUM") as ps_ot, \
             tc.tile_pool(name="ps_m", bufs=2, space="PSUM") as ps_m:
            for s in range(nsb):
                blo = 4 * s
                bhi = min(4 * s + 4, nb)
                w = bhi - blo
                # scores for up to 4 blocks in one psum bank -> one exp
                sc = ps_sc.tile([BCAP, w * BCAP], fp32, tag="sc")
                for j in range(w):
                    c = (blo + j) * BCAP
                    nc.tensor.matmul(sc[:, j * BCAP:(j + 1) * BCAP],
                                     sb_xsT[:, c:c + BCAP],
                                     sb_xtT[:, c:c + BCAP],
                                     start=True, stop=True)
                et = epool.tile([BCAP, w * BCAP], bf16, tag="E")
                nc.scalar.activation(out=et, in_=sc, func=AF.Exp)

                # process blocks in pairs: one psum V-out tile + one
                # broadcast-divide + one residual add per pair
                for j0 in range(0, w, 2):
                    g = min(2, w - j0)
                    o = ps_ot.tile([BCAP, g * 2 * VW], fp32, tag="ot")
                    for j in range(j0, j0 + g):
                        b = blo + j
                        va = b * VW
                        q = (j - j0) * 2 * VW
                        # E^T for the src side: ST = SS^T
                        etT = tpool.tile([BCAP, BCAP], bf16, tag="etT")
                        nc.sync.dma_start_transpose(
                            out=etT, in_=et[:, j * BCAP:(j + 1) * BCAP])
                        nc.tensor.matmul(o[:, q:q + VW], etT,
                                         sb_vt[:, va:va + VW],
                                         start=True, stop=True)
                        nc.tensor.matmul(o[:, q + VW:q + 2 * VW],
                                         et[:, j * BCAP:(j + 1) * BCAP],
                                         sb_vs[:, va:va + VW],
                                         start=True, stop=True)

                    # evict + normalize: er_raw = o[:, :64] * (1/rowsum)
                    # (reciprocal also evicts the psum rowsums to sbuf so the
                    # multiply has a single psum operand)
                    o4 = o.rearrange("p (g s v) -> p g s v", s=2, v=VW)
                    rc = rpool.tile([BCAP, g * 2], fp32, tag="rc")
                    nc.vector.reciprocal(
                        out=rc.rearrange("p (g s v) -> p g s v", s=2, v=1),
                        in_=o4[:, :, :, D:D + 1])
                    er_raw = rpool.tile([BCAP, g * 2 * D], fp16, tag="eraw")
                    er4 = er_raw.rearrange("p (g s v) -> p g s v", s=2, v=D)
                    nc.vector.tensor_tensor(
                        out=er4, in0=o4[:, :, :, 0:D],
                        in1=rc.rearrange("p (g s v) -> p g s v", s=2, v=1)
                            .broadcast_to([BCAP, g, 2, D]),
                        op=ALU.mult)
                    # + x residual (sbuf only -> Pool engine)
                    er = rpool.tile([BCAP, g * 2 * D], fp16, tag="er")
                    rc = (blo + j0) * 2 * D
                    nc.gpsimd.tensor_tensor(
                        out=er, in0=er_raw,
                        in1=sb_vres[:, rc:rc + g * 2 * D], op=ALU.add)
                    # feature-major eviction straight into the MLP input
                    for j in range(j0, j0 + g):
                        c = (blo + j) * BCAP
                        jj = (j - j0) * BCAP
                        nc.sync.dma_start_transpose(
                            out=sb_eT[:, c:c + BCAP],
                            in_=er[:, jj:jj + BCAP])

            # ---- packed MLP over [128, rows] ----
            for c in range(0, rows, mlp_ch):
                hp = ps_m.tile([2 * D, mlp_ch], fp32, tag="m")
                nc.tensor.matmul(hp, sb_w1, sb_eT[:, c:c + mlp_ch],
                                 start=True, stop=True)
                nc.scalar.activation(out=sb_h[:, c:c + mlp_ch], in_=hp,
                                     func=AF.Relu, bias=sb_b1, scale=1.0)
            for c in range(0, rows, mlp_ch):
                op2 = ps_m.tile([2 * D, mlp_ch], fp32, tag="m")
                nc.tensor.matmul(op2, sb_w2, sb_h[:, c:c + mlp_ch],
                                 start=True, stop=True)
                nc.vector.scalar_tensor_tensor(
                    out=sb_out[:, c:c + mlp_ch], in0=op2, scalar=sb_b2,
                    in1=sb_eT[:, c:c + mlp_ch], op0=ALU.add, op1=ALU.add)
            nc.sync.dma_start(out=outp[:, :], in_=sb_out)

    nc.compile()
    return nc


def _pack_blocks(cnt_s, cnt_t):
    """Pair graphs into 128-row blocks. Returns list of blocks, each a
    list of (graph_id, row_offset)."""
    n = np.maximum(cnt_s, cnt_t)
    order = np.argsort(n, kind="stable")
    lo, hi = 0, len(order) - 1
    blocks = []
    while lo <= hi:
        g_hi = order[hi]
        if lo < hi and n[order[lo]] + n[g_hi] <= BCAP:
            g_lo = order[lo]
            blocks.append([(int(g_hi), 0), (int(g_lo), int(n[g_hi]))])
            lo += 1
        else:
            blocks.append([(int(g_hi), 0)])
        hi -= 1
    return blocks


def _shard_inputs(x_src, batch_src, x_tar, batch_tar, w1, b1, w2, b2):
    bs = np.asarray(batch_src).astype(np.int64)
    bt = np.asarray(batch_tar).astype(np.int64)
    xs = np.asarray(x_src, dtype=np.float32)
    xt = np.asarray(x_tar, dtype=np.float32)

    bnd_s = np.searchsorted(bs, np.arange(G + 1))
    bnd_t = np.searchsorted(bt, np.arange(G + 1))
    cnt_s = np.diff(bnd_s)
    cnt_t = np.diff(bnd_t)
    if np.maximum(cnt_s, cnt_t).max(initial=0) > BCAP:
        return None, None, (bnd_s, bnd_t, cnt_s, cnt_t)

    blocks = _pack_blocks(cnt_s, cnt_t)
    nb = (len(blocks) + N_CORES - 1) // N_CORES
    rows = nb * BCAP
    # deal blocks to cores round-robin
    core_blocks = [[] for _ in range(N_CORES)]
    for i, blk in enumerate(blocks):
        core_blocks[i % N_CORES].append(blk)

    w1a = np.asarray(w1, dtype=np.float32)
    w2a = np.asarray(w2, dtype=np.float32)
    b1a = np.asarray(b1, dtype=np.float32).reshape(D)
    b2a = np.asarray(b2, dtype=np.float32).reshape(D)
    w1bd = np.zeros((2 * D, 2 * D), dtype=np.float16)
    w2bd = np.zeros((2 * D, 2 * D), dtype=np.float16)
    w1bd[:D, :D] = w1a; w1bd[D:, D:] = w1a
    w2bd[:D, :D] = w2a; w2bd[D:, D:] = w2a
    b1bd = np.concatenate([b1a, b1a]).reshape(2 * D, 1)
    b2bd = np.concatenate([b2a, b2a]).reshape(2 * D, 1)

    to_bf16 = _bf16_caster()

    in_maps = []
    placement = []   # per core: list of (g, row_off_in_core_img)
    for c in range(N_CORES):
        blks = core_blocks[c]
        xs_img = np.zeros((CD, rows), dtype=np.float16)
        xt_img = np.zeros((CD, rows), dtype=np.float16)
        vres_img = np.zeros((BCAP, nb * 2 * D), dtype=np.float16)
        vs_img = np.zeros((BCAP, nb * VW), dtype=np.float32)
        vt_img = np.zeros((BCAP, nb * VW), dtype=np.float32)
        vs_img[:, VW - 1::VW] = 1.0   # mask column: ones everywhere
        vt_img[:, VW - 1::VW] = 1.0
        # ones bias row on ALL query rows (incl. padding): pad queries then
        # score -NEG against every key, so their exp(~0) rows don't pollute
        # the transposed-side rowsums through the all-ones mask column
        xs_img[D, :] = 1.0
        place = []
        for bi, blk in enumerate(blks):
            col = bi * BCAP
            for gi, (g, off) in enumerate(blk):
                ns, nt = cnt_s[g], cnt_t[g]
                sig = 1.0 if gi == 0 else -1.0
                xs_img[:D, col + off:col + off + ns] = xs[bnd_s[g]:bnd_s[g + 1]].T
                xt_img[:D, col + off:col + off + nt] = xt[bnd_t[g]:bnd_t[g + 1]].T
                # bias rows: score' = q.k + 1_q*(-NEG)_k + sig_q*(NEG*sig)_k
                xs_img[D + 1, col + off:col + off + ns] = sig
                xt_img[D, col + off:col + off + nt] = -NEG
                xt_img[D + 1, col + off:col + off + nt] = NEG * sig
                vs_img[off:off + ns, bi * VW:bi * VW + D] = xs[bnd_s[g]:bnd_s[g + 1]]
                vt_img[off:off + nt, bi * VW:bi * VW + D] = xt[bnd_t[g]:bnd_t[g + 1]]
                vres_img[off:off + ns, bi * 2 * D:bi * 2 * D + D] = \
                    xs[bnd_s[g]:bnd_s[g + 1]]
                vres_img[off:off + nt, bi * 2 * D + D:(bi + 1) * 2 * D] = \
                    xt[bnd_t[g]:bnd_t[g + 1]]
                place.append((int(g), col + off))
        # pad columns of real (non-empty) blocks still get the -NEG bias so
        # their exp is ~0; zero-filled xt bias rows already handle empty
        # blocks (rowsum = 128 from the ones mask -> no div by zero)
        for bi in range(len(blks)):
            col = bi * BCAP
            m = xt_img[D, col:col + BCAP] == 0.0
            xt_img[D, col:col + BCAP][m] = -NEG
        in_maps.append({
            "xsT": xs_img,
            "xtT": xt_img,
            "vs": to_bf16(vs_img),
            "vt": to_bf16(vt_img),
            "vres": vres_img,
            "w1bd": w1bd, "b1bd": b1bd, "w2bd": w2bd, "b2bd": b2bd,
        })
        placement.append(place)
    meta = (bnd_s, bnd_t, cnt_s, cnt_t, placement, nb)
    return in_maps, nb, meta


def _bf16_caster():
    import ml_dtypes
    return lambda a: a.astype(ml_dtypes.bfloat16)


def _numpy_fallback(x_src, batch_src, x_tar, batch_tar, w1, b1, w2, b2):
    bs = np.asarray(batch_src); bt = np.asarray(batch_tar)
    xs = np.asarray(x_src, dtype=np.float64); xt = np.asarray(x_tar, dtype=np.float64)
    mask = bs[:, None] == bt[None, :]

    def attend(q, kv, m):
        s = np.where(m, q @ kv.T, -1.0e9)
        s = s - s.max(axis=1, keepdims=True)
        e = np.exp(s)
        a = e / e.sum(axis=1, keepdims=True)
        out = a @ kv + q
        return np.where(m.any(axis=1, keepdims=True), out, 0.0)

    def mlp(x):
        return np.maximum(x @ w1 + b1, 0.0) @ w2 + b2 + x

    es = mlp(attend(xs, xt, mask))
    et = mlp(attend(xt, xs, mask.T))
    return et.astype(np.float32), es.astype(np.float32)


def kernel(x_src, batch_src, x_tar, batch_tar, w1, b1, w2, b2):
    in_maps, nb, meta = _shard_inputs(
        x_src, batch_src, x_tar, batch_tar, w1, b1, w2, b2)
    if in_maps is None:  # a graph overflowed BCAP; never happens for spec data
        return _numpy_fallback(
            x_src, batch_src, x_tar, batch_tar, w1, b1, w2, b2)
    bnd_s, bnd_t, cnt_s, cnt_t, placement, nb = meta
    rows = nb * BCAP
    mlp_ch = rows // 4
    assert rows % 4 == 0 and mlp_ch <= 512

    import os
    from concourse import bass_utils
    key = (nb, mlp_ch)
    if key not in _PROGRAM_CACHE:
        _PROGRAM_CACHE[key] = _build_program(nb, mlp_ch)
    nc = _PROGRAM_CACHE[key]
    trace = bool(os.environ.get("KERNEL_TRACE"))
    res = bass_utils.run_bass_kernel_spmd(
        nc, in_maps, core_ids=list(range(N_CORES)), trace=trace)
    _PROGRAM_CACHE["last_result"] = res

    # rows whose graph has no counterpart: reference yields mlp(0)
    w1a = np.asarray(w1, np.float32); b1a = np.asarray(b1, np.float32)
    w2a = np.asarray(w2, np.float32); b2a = np.asarray(b2, np.float32)
    mlp0 = np.maximum(b1a, 0.0) @ w2a + b2a

    embed_src = np.zeros((N_NODES, D), dtype=np.float32)
    embed_tar = np.zeros((N_NODES, D), dtype=np.float32)
    for c in range(N_CORES):
        op = np.asarray(res.results[c]["outp"]).astype(np.float32)
        for g, off in placement[c]:
            ns, nt = cnt_s[g], cnt_t[g]
            if ns > 0:
                embed_src[bnd_s[g]:bnd_s[g] + ns] = (
                    op[0:D, off:off + ns].T if nt > 0 else mlp0)
            if nt > 0:
                embed_tar[bnd_t[g]:bnd_t[g] + nt] = (
                    op[D:2 * D, off:off + nt].T if ns > 0 else mlp0)
    return embed_tar, embed_src


# revision 15
# speedup vs baseline: 2.1990x; 1.6721x over previous
"""Block-diagonal cross-attention + MLP for trn2, 8-core data-parallel.

v3: graphs bin-packed in pairs into 128-row blocks (NB blocks/core).
Cross-graph + padding masking is folded into the score matmul via two
extra contraction rows (graph-code sigma in {+1,-1}): score' = q.k
- 25*(1 - sigma_q*sigma_k), so cross-graph pairs get -50 and padded
columns -25 -> exp ~ 0.  Only exp(SS) is computed on ACT; E^T comes
from a DMA transpose (ST = SS^T exactly).  V-matmul eviction fuses
normalize (divide by the mask-column rowsum) and the +x residual via
scalar_tensor_tensor on DVE/Pool.  er is DMA-transposed straight into
the feature-major MLP input.  MLP uses block-diagonal [128,128]
weights to do both sides at once; residual+bias fused into the final
eviction.  dtypes: fp16 images/weights everywhere except E/V (bf16,
exp can reach e^46 which overflows fp16).
Output: [128, NB*128] fp16 per core; host scatters per graph.
"""

from contextlib import ExitStack

import numpy as np

N_NODES = 8192
D = 64
G = 128
N_CORES = 8
BCAP = 128                  # rows per block
CD = D + 2                  # contraction rows incl. bias rows
VW = D + 1                  # v image width incl. mask column
NEG = 25.0                  # pad bias; cross-graph pairs get -2*NEG

_PROGRAM_CACHE = {}


def _build_program(nb, mlp_ch):
    import concourse.bass as bass
    import concourse.tile as tile
    from concourse import bacc, mybir

    fp32 = mybir.dt.float32
    fp16 = mybir.dt.float16
    bf16 = mybir.dt.bfloat16
    rows = nb * BCAP
    nsb = (nb + 1) // 2           # superblocks of up to 2 blocks
    nc = bacc.Bacc("TRN2", target_bir_lowering=False, debug=False)

    xsT = nc.declare_dram_parameter("xsT", [CD, rows], fp16, isOutput=False)
    xtT = nc.declare_dram_parameter("xtT", [CD, rows], fp16, isOutput=False)
    vs = nc.declare_dram_parameter("vs", [BCAP, nb * VW], bf16, isOutput=False)
    vt = nc.declare_dram_parameter("vt", [BCAP, nb * VW], bf16, isOutput=False)
    vres = nc.declare_dram_parameter("vres", [BCAP, nb * 2 * D], fp16,
                                     isOutput=False)
    w1bd = nc.declare_dram_parameter("w1bd", [2 * D, 2 * D], fp16, isOutput=False)
    b1bd = nc.declare_dram_parameter("b1bd", [2 * D, 1], fp32, isOutput=False)
    w2bd = nc.declare_dram_parameter("w2bd", [2 * D, 2 * D], fp16, isOutput=False)
    b2bd = nc.declare_dram_parameter("b2bd", [2 * D, 1], fp32, isOutput=False)
    idh = nc.declare_dram_parameter("idh", [BCAP, BCAP], fp16, isOutput=False)
    idb = nc.declare_dram_parameter("idb", [BCAP, BCAP], bf16, isOutput=False)
    outp = nc.declare_dram_parameter("outp", [2 * D, rows], fp16, isOutput=True)

    AF = mybir.ActivationFunctionType
    ALU = mybir.AluOpType

    with tile.TileContext(nc) as tc, ExitStack() as ctx:
        singles = ctx.enter_context(tc.tile_pool(name="singles", bufs=1))
        epool = ctx.enter_context(tc.tile_pool(name="epool", bufs=3))
        tpool = ctx.enter_context(tc.tile_pool(name="tpool", bufs=3))
        rpool = ctx.enter_context(tc.tile_pool(name="rpool", bufs=3))

        sb_xsT = singles.tile([CD, rows], fp16, tag="xsT")
        sb_xtT = singles.tile([CD, rows], fp16, tag="xtT")
        sb_vs = singles.tile([BCAP, nb * VW], bf16, tag="vs")
        sb_vt = singles.tile([BCAP, nb * VW], bf16, tag="vt")
        sb_vres = singles.tile([BCAP, nb * 2 * D], fp16, tag="vres")
        sb_w1 = singles.tile([2 * D, 2 * D], fp16, tag="w1")
        sb_b1 = singles.tile([2 * D, 1], fp32, tag="b1")
        sb_w2 = singles.tile([2 * D, 2 * D], fp16, tag="w2")
        sb_b2 = singles.tile([2 * D, 1], fp32, tag="b2")
        sb_eT = singles.tile([2 * D, rows], fp16, tag="eT")
        sb_h = singles.tile([2 * D, rows], fp16, tag="h")
        sb_out = singles.tile([2 * D, rows], fp16, tag="out")
        sb_idh = singles.tile([BCAP, BCAP], fp16, tag="idh")
        sb_idb = singles.tile([BCAP, BCAP], bf16, tag="idb")

        nc.sync.dma_start(out=sb_xsT, in_=xsT[:, :])
        nc.sync.dma_start(out=sb_xtT, in_=xtT[:, :])
        nc.sync.dma_start(out=sb_vs, in_=vs[:, :])
        nc.sync.dma_start(out=sb_vt, in_=vt[:, :])
        nc.sync.dma_start(out=sb_vres, in_=vres[:, :])
        nc.sync.dma_start(out=sb_w1, in_=w1bd[:, :])
        nc.sync.dma_start(out=sb_b1, in_=b1bd[:, :])
        nc.sync.dma_start(out=sb_w2, in_=w2bd[:, :])
        nc.sync.dma_start(out=sb_b2, in_=b2bd[:, :])
        nc.sync.dma_start(out=sb_idh, in_=idh[:, :])
        nc.sync.dma_start(out=sb_idb, in_=idb[:, :])

        with tc.tile_pool(name="ps_sc", bufs=2, space="PSUM") as ps_sc, \
             tc.tile_pool(name="ps_ot", bufs=3, space="PSUM") as ps_ot, \
             tc.tile_pool(name="ps_m", bufs=2, space="PSUM") as ps_m, \
             tc.tile_pool(name="ps_tp", bufs=4, space="PSUM") as ps_tp:
            for s in range(nsb):
                blo = 2 * s
                g = min(2, nb - blo)
                # scores for g blocks, BOTH orientations [SS_j|ST_j] in one
                # psum bank -> one exp; ST computed directly on PE so no
                # E^T transpose is ever needed
                sc = ps_sc.tile([BCAP, g * 2 * BCAP], fp32, tag="sc")
                for j in range(g):
                    c = (blo + j) * BCAP
                    q = j * 2 * BCAP
                    nc.tensor.matmul(sc[:, q:q + BCAP],
                                     sb_xsT[:, c:c + BCAP],
                                     sb_xtT[:, c:c + BCAP],
                                     start=True, stop=True)
                    nc.tensor.matmul(sc[:, q + BCAP:q + 2 * BCAP],
                                     sb_xtT[:, c:c + BCAP],
                                     sb_xsT[:, c:c + BCAP],
                                     start=True, stop=True)
                et = epool.tile([BCAP, g * 2 * BCAP], bf16, tag="E")
                nc.scalar.activation(out=et, in_=sc, func=AF.Exp)

                if True:
                    j0 = 0
                    o = ps_ot.tile([BCAP, g * 2 * VW], fp32, tag="ot")
                    for j in range(j0, j0 + g):
                        b = blo + j
                        va = b * VW
                        q = (j - j0) * 2 * VW
                        e_ss = et[:, j * 2 * BCAP:j * 2 * BCAP + BCAP]
                        e_st = et[:, j * 2 * BCAP + BCAP:(j + 1) * 2 * BCAP]
                        nc.tensor.matmul(o[:, q:q + VW], e_st,
                                         sb_vt[:, va:va + VW],
                                         start=True, stop=True)
                        nc.tensor.matmul(o[:, q + VW:q + 2 * VW], e_ss,
                                         sb_vs[:, va:va + VW],
                                         start=True, stop=True)

                    # evict + normalize: er_raw = o[:, :64] * (1/rowsum)
                    # (reciprocal also evicts the psum rowsums to sbuf so the
                    # multiply has a single psum operand)
                    o4 = o.rearrange("p (g s v) -> p g s v", s=2, v=VW)
                    rc = rpool.tile([BCAP, g * 2], fp32, tag="rc")
                    nc.vector.reciprocal(
                        out=rc.rearrange("p (g s v) -> p g s v", s=2, v=1),
                        in_=o4[:, :, :, D:D + 1])
                    er_raw = rpool.tile([BCAP, g * 2 * D], fp16, tag="eraw")
                    er4 = er_raw.rearrange("p (g s v) -> p g s v", s=2, v=D)
                    nc.vector.tensor_tensor(
                        out=er4, in0=o4[:, :, :, 0:D],
                        in1=rc.rearrange("p (g s v) -> p g s v", s=2, v=1)
                            .broadcast_to([BCAP, g, 2, D]),
                        op=ALU.mult)
                    # + x residual (sbuf only -> Pool engine)
                    er = rpool.tile([BCAP, g * 2 * D], fp16, tag="er")
                    rc = (blo + j0) * 2 * D
                    nc.gpsimd.tensor_tensor(
                        out=er, in0=er_raw,
                        in1=sb_vres[:, rc:rc + g * 2 * D], op=ALU.add)
                    # feature-major eviction straight into the MLP input
                    for j in range(j0, j0 + g):
                        c = (blo + j) * BCAP
                        jj = (j - j0) * BCAP
                        eng = nc.sync if (blo + j) % 2 == 0 else nc.scalar
                        eng.dma_start_transpose(
                            out=sb_eT[:, c:c + BCAP],
                            in_=er[:, jj:jj + BCAP])

            # ---- packed MLP over [128, rows] ----
            for c in range(0, rows, mlp_ch):
                hp = ps_m.tile([2 * D, mlp_ch], fp32, tag="m")
                nc.tensor.matmul(hp, sb_w1, sb_eT[:, c:c + mlp_ch],
                                 start=True, stop=True)
                nc.scalar.activation(out=sb_h[:, c:c + mlp_ch], in_=hp,
                                     func=AF.Relu, bias=sb_b1, scale=1.0)
            for c in range(0, rows, mlp_ch):
                op2 = ps_m.tile([2 * D, mlp_ch], fp32, tag="m")
                nc.tensor.matmul(op2, sb_w2, sb_h[:, c:c + mlp_ch],
                                 start=True, stop=True)
                nc.vector.scalar_tensor_tensor(
                    out=sb_out[:, c:c + mlp_ch], in0=op2, scalar=sb_b2,
                    in1=sb_eT[:, c:c + mlp_ch], op0=ALU.add, op1=ALU.add)
            nc.sync.dma_start(out=outp[:, :], in_=sb_out)

    nc.compile()
    return nc


def _pack_blocks(cnt_s, cnt_t):
    """Pair graphs into 128-row blocks. Returns list of blocks, each a
    list of (graph_id, row_offset)."""
    n = np.maximum(cnt_s, cnt_t)
    order = np.argsort(n, kind="stable")
    lo, hi = 0, len(order) - 1
    blocks = []
    while lo <= hi:
        g_hi = order[hi]
        if lo < hi and n[order[lo]] + n[g_hi] <= BCAP:
            g_lo = order[lo]
            blocks.append([(int(g_hi), 0), (int(g_lo), int(n[g_hi]))])
            lo += 1
        else:
            blocks.append([(int(g_hi), 0)])
        hi -= 1
    return blocks


def _shard_inputs(x_src, batch_src, x_tar, batch_tar, w1, b1, w2, b2):
    bs = np.asarray(batch_src).astype(np.int64)
    bt = np.asarray(batch_tar).astype(np.int64)
    xs = np.asarray(x_src, dtype=np.float32)
    xt = np.asarray(x_tar, dtype=np.float32)

    bnd_s = np.searchsorted(bs, np.arange(G + 1))
    bnd_t = np.searchsorted(bt, np.arange(G + 1))
    cnt_s = np.diff(bnd_s)
    cnt_t = np.diff(bnd_t)
    if np.maximum(cnt_s, cnt_t).max(initial=0) > BCAP:
        return None, None, (bnd_s, bnd_t, cnt_s, cnt_t)

    blocks = _pack_blocks(cnt_s, cnt_t)
    nb = (len(blocks) + N_CORES - 1) // N_CORES
    rows = nb * BCAP
    # deal blocks to cores round-robin
    core_blocks = [[] for _ in range(N_CORES)]
    for i, blk in enumerate(blocks):
        core_blocks[i % N_CORES].append(blk)

    w1a = np.asarray(w1, dtype=np.float32)
    w2a = np.asarray(w2, dtype=np.float32)
    b1a = np.asarray(b1, dtype=np.float32).reshape(D)
    b2a = np.asarray(b2, dtype=np.float32).reshape(D)
    w1bd = np.zeros((2 * D, 2 * D), dtype=np.float16)
    w2bd = np.zeros((2 * D, 2 * D), dtype=np.float16)
    w1bd[:D, :D] = w1a; w1bd[D:, D:] = w1a
    w2bd[:D, :D] = w2a; w2bd[D:, D:] = w2a
    b1bd = np.concatenate([b1a, b1a]).reshape(2 * D, 1)
    b2bd = np.concatenate([b2a, b2a]).reshape(2 * D, 1)

    to_bf16 = _bf16_caster()

    in_maps = []
    placement = []   # per core: list of (g, row_off_in_core_img)
    for c in range(N_CORES):
        blks = core_blocks[c]
        xs_img = np.zeros((CD, rows), dtype=np.float16)
        xt_img = np.zeros((CD, rows), dtype=np.float16)
        vres_img = np.zeros((BCAP, nb * 2 * D), dtype=np.float16)
        vs_img = np.zeros((BCAP, nb * VW), dtype=np.float32)
        vt_img = np.zeros((BCAP, nb * VW), dtype=np.float32)
        vs_img[:, VW - 1::VW] = 1.0   # mask column: ones everywhere
        vt_img[:, VW - 1::VW] = 1.0
        # ones bias row on ALL query rows (incl. padding): pad queries then
        # score -NEG against every key, so their exp(~0) rows don't pollute
        # the transposed-side rowsums through the all-ones mask column
        xs_img[D, :] = 1.0
        place = []
        for bi, blk in enumerate(blks):
            col = bi * BCAP
            for gi, (g, off) in enumerate(blk):
                ns, nt = cnt_s[g], cnt_t[g]
                sig = 1.0 if gi == 0 else -1.0
                xs_img[:D, col + off:col + off + ns] = xs[bnd_s[g]:bnd_s[g + 1]].T
                xt_img[:D, col + off:col + off + nt] = xt[bnd_t[g]:bnd_t[g + 1]].T
                # bias rows: score' = q.k + 1_q*(-NEG)_k + sig_q*(NEG*sig)_k
                xs_img[D + 1, col + off:col + off + ns] = sig
                xt_img[D, col + off:col + off + nt] = -NEG
                xt_img[D + 1, col + off:col + off + nt] = NEG * sig
                vs_img[off:off + ns, bi * VW:bi * VW + D] = xs[bnd_s[g]:bnd_s[g + 1]]
                vt_img[off:off + nt, bi * VW:bi * VW + D] = xt[bnd_t[g]:bnd_t[g + 1]]
                vres_img[off:off + ns, bi * 2 * D:bi * 2 * D + D] = \
                    xs[bnd_s[g]:bnd_s[g + 1]]
                vres_img[off:off + nt, bi * 2 * D + D:(bi + 1) * 2 * D] = \
                    xt[bnd_t[g]:bnd_t[g + 1]]
                place.append((int(g), col + off))
        # pad columns of real (non-empty) blocks still get the -NEG bias so
        # their exp is ~0; zero-filled xt bias rows already handle empty
        # blocks (rowsum = 128 from the ones mask -> no div by zero)
        for bi in range(len(blks)):
            col = bi * BCAP
            m = xt_img[D, col:col + BCAP] == 0.0
            xt_img[D, col:col + BCAP][m] = -NEG
        ident = np.eye(BCAP, dtype=np.float32)
        in_maps.append({
            "xsT": xs_img,
            "xtT": xt_img,
            "vs": to_bf16(vs_img),
            "vt": to_bf16(vt_img),
            "vres": vres_img,
            "idh": ident.astype(np.float16),
            "idb": to_bf16(ident),
            "w1bd": w1bd, "b1bd": b1bd, "w2bd": w2bd, "b2bd": b2bd,
        })
        placement.append(place)
    meta = (bnd_s, bnd_t, cnt_s, cnt_t, placement, nb)
    return in_maps, nb, meta


def _bf16_caster():
    import ml_dtypes
    return lambda a: a.astype(ml_dtypes.bfloat16)


def _numpy_fallback(x_src, batch_src, x_tar, batch_tar, w1, b1, w2, b2):
    bs = np.asarray(batch_src); bt = np.asarray(batch_tar)
    xs = np.asarray(x_src, dtype=np.float64); xt = np.asarray(x_tar, dtype=np.float64)
    mask = bs[:, None] == bt[None, :]

    def attend(q, kv, m):
        s = np.where(m, q @ kv.T, -1.0e9)
        s = s - s.max(axis=1, keepdims=True)
        e = np.exp(s)
        a = e / e.sum(axis=1, keepdims=True)
        out = a @ kv + q
        return np.where(m.any(axis=1, keepdims=True), out, 0.0)

    def mlp(x):
        return np.maximum(x @ w1 + b1, 0.0) @ w2 + b2 + x

    es = mlp(attend(xs, xt, mask))
    et = mlp(attend(xt, xs, mask.T))
    return et.astype(np.float32), es.astype(np.float32)


def kernel(x_src, batch_src, x_tar, batch_tar, w1, b1, w2, b2):
    in_maps, nb, meta = _shard_inputs(
        x_src, batch_src, x_tar, batch_tar, w1, b1, w2, b2)
    if in_maps is None:  # a graph overflowed BCAP; never happens for spec data
        return _numpy_fallback(
            x_src, batch_src, x_tar, batch_tar, w1, b1, w2, b2)
    bnd_s, bnd_t, cnt_s, cnt_t, placement, nb = meta
    rows = nb * BCAP
    mlp_ch = rows // 4
    assert rows % 4 == 0 and mlp_ch <= 512

    import os
    from concourse import bass_utils
    key = (nb, mlp_ch)
    if key not in _PROGRAM_CACHE:
        _PROGRAM_CACHE[key] = _build_program(nb, mlp_ch)
    nc = _PROGRAM_CACHE[key]
    trace = bool(os.environ.get("KERNEL_TRACE"))
    res = bass_utils.run_bass_kernel_spmd(
        nc, in_maps, core_ids=list(range(N_CORES)), trace=trace)
    _PROGRAM_CACHE["last_result"] = res

    # rows whose graph has no counterpart: reference yields mlp(0)
    w1a = np.asarray(w1, np.float32); b1a = np.asarray(b1, np.float32)
    w2a = np.asarray(w2, np.float32); b2a = np.asarray(b2, np.float32)
    mlp0 = np.maximum(b1a, 0.0) @ w2a + b2a

    embed_src = np.zeros((N_NODES, D), dtype=np.float32)
    embed_tar = np.zeros((N_NODES, D), dtype=np.float32)
    for c in range(N_CORES):
        op = np.asarray(res.results[c]["outp"]).astype(np.float32)
        for g, off in placement[c]:
            ns, nt = cnt_s[g], cnt_t[g]
            if ns > 0:
                embed_src[bnd_s[g]:bnd_s[g] + ns] = (
                    op[0:D, off:off + ns].T if nt > 0 else mlp0)
            if nt > 0:
                embed_tar[bnd_t[g]:bnd_t[g] + nt] = (
                    op[D:2 * D, off:off + nt].T if ns > 0 else mlp0)
    return embed_tar, embed_src
